# revision 8
# baseline (speedup 1.0000x reference)
"""Trainium2 Bass kernel for the AnomalyBlock problem.

Strategy: data-parallel over batch B=8 (one batch element per NeuronCore).
Each core runs the full attention block for its batch element:
  q/k/v/o projections (float32r matmuls, biases folded in as K=1 rank-1
  matmuls against a ones-row), scores in BOTH orientations ([l,s] for the
  normalized attention output, [s,l] for the A@V contraction since the PE
  contracts over the partition dim), softmax via ACT exp with accum_out
  (row sums for free) + DVE per-partition normalize.
prior_attn is batch-broadcast, so only [H,L,L] is unique; its rows are
sharded across the 8 cores (core c computes rows c*128..c*128+127 for all
heads) and the batch dim is broadcast on the host at gather time.
"""
import sys
import numpy as np

sys.path.insert(0, "/opt/trn_rl_repo")

B, L, D, H, DK = 8, 1024, 256, 8, 32
N_CORES = 8
SCALE = 1.0 / np.sqrt(DK)

_CACHE = {}


def _build():
    import concourse.bacc as bacc
    import concourse.mybir as mybir
    import concourse.tile as tile

    F32 = mybir.dt.float32
    F32R = mybir.dt.float32r
    AF = mybir.ActivationFunctionType
    ALU = mybir.AluOpType

    nc = bacc.Bacc("TRN2", target_bir_lowering=False, debug=False,
                   num_devices=N_CORES)

    x_d = nc.dram_tensor("x", [L, D], F32, kind="ExternalInput")
    w_d = {k: nc.dram_tensor(f"W{k}", [D, D], F32, kind="ExternalInput")
           for k in "qkvo"}
    b_d = {k: nc.dram_tensor(f"b{k}", [1, D], F32, kind="ExternalInput")
           for k in "qkvo"}
    sig_d = nc.dram_tensor("sig", [1, H], F32, kind="ExternalInput")
    dist2n_d = nc.dram_tensor("dist2n", [128, L], F32, kind="ExternalInput")
    eye_d = nc.dram_tensor("eye", [128, 128], F32, kind="ExternalInput")
    ones_d = nc.dram_tensor("ones", [1, L], F32, kind="ExternalInput")
    bsel_d = nc.dram_tensor("bsel", [8, 2, 128], F32, kind="ExternalInput")

    attn_d = nc.dram_tensor("attn", [H, L, L], F32, kind="ExternalOutput")
    prior_d = nc.dram_tensor("prior", [H, 128, L], F32, kind="ExternalOutput")
    out_d = nc.dram_tensor("out", [L, D], F32, kind="ExternalOutput")

    with tile.TileContext(nc) as tc:
        with (
            tc.tile_pool(name="const", bufs=1) as cpool,
            tc.tile_pool(name="attnp", bufs=6) as pattn,
            tc.tile_pool(name="eTp", bufs=6) as peT,
            tc.tile_pool(name="tinyp", bufs=8) as ptiny,
            tc.tile_pool(name="mm2", bufs=2, space="PSUM") as pmm2,
            tc.tile_pool(name="avp", bufs=2, space="PSUM") as pav,
            tc.tile_pool(name="smallp", bufs=2, space="PSUM") as psmall,
        ):
            # ---- Phase 0: input DMAs ----
            x_sb = cpool.tile([128, 8, D], F32)
            nc.sync.dma_start(x_sb[:], x_d.rearrange("(a p) d -> p a d", p=128))
            w_sb = {}
            for k in "qkvo":
                w_sb[k] = cpool.tile([128, 2, D], F32R, tag=f"W{k}", name=f"W{k}_sb")
                nc.gpsimd.dma_start(
                    w_sb[k][:], w_d[k].rearrange("(k p) n -> p k n", p=128))
            b_sb = {}
            for k in "qkvo":
                b_sb[k] = cpool.tile([1, D], F32R, tag=f"b{k}", name=f"b{k}_sb")
                nc.gpsimd.dma_start(b_sb[k][:], b_d[k][:])
            sig_sb = cpool.tile([1, H], F32)
            nc.sync.dma_start(sig_sb[:], sig_d[:])
            dist2n_sb = cpool.tile([128, L], F32)
            nc.sync.dma_start(dist2n_sb[:], dist2n_d[:])
            eye_sb = cpool.tile([128, 128], F32)
            nc.sync.dma_start(eye_sb[:], eye_d[:])
            ones_f = cpool.tile([1, L], F32)
            nc.sync.dma_start(ones_f[:], ones_d[:])
            ones_r = cpool.tile([1, L], F32R)
            nc.gpsimd.dma_start(ones_r[:], ones_d[:])
            bsel_sb = cpool.tile([8, 2, 128], F32R)
            nc.gpsimd.dma_start(bsel_sb[:], bsel_d[:])

            # ---- Phase 1: xT = x transposed, [d, l] layout, f32r ----
            xT = [cpool.tile([128, L], F32R, tag=f"xT{dc}", name=f"xT{dc}") for dc in range(2)]
            for li in range(8):
                for dc in range(2):
                    ps = psmall.tile([128, 128], F32, tag="small", name="ps")
                    nc.tensor.transpose(
                        ps[:], x_sb[:, li, dc * 128:(dc + 1) * 128], eye_sb[:])
                    nc.vector.tensor_copy(
                        xT[dc][:, li * 128:(li + 1) * 128], ps[:])

            # ---- Phase 2: projections ----
            # QT/KT: [dout, l] layout (head-major on partitions), f32r
            QT = [cpool.tile([128, L], F32R, tag=f"QT{dc}", name=f"QT{dc}") for dc in range(2)]
            KT = [cpool.tile([128, L], F32R, tag=f"KT{dc}", name=f"KT{dc}") for dc in range(2)]
            for name, wk, bk, dst in (("q", "q", "q", QT), ("k", "k", "k", KT)):
                for dc in range(2):
                    for lh in range(2):
                        ps = psmall.tile([128, 512], F32, tag="small", name="ps")
                        for kc in range(2):
                            nc.tensor.matmul(
                                ps[:],
                                w_sb[wk][:, kc, dc * 128:(dc + 1) * 128],
                                xT[kc][:, lh * 512:(lh + 1) * 512],
                                start=(kc == 0), stop=False)
                        nc.tensor.matmul(
                            ps[:],
                            b_sb[bk][0:1, dc * 128:(dc + 1) * 128],
                            ones_r[0:1, lh * 512:(lh + 1) * 512],
                            start=False, stop=True)
                        nc.scalar.copy(
                            dst[dc][:, lh * 512:(lh + 1) * 512], ps[:])
            # V: natural [s, dout] layout, f32r
            V_sb = cpool.tile([128, 8, D], F32R)
            for si in range(8):
                ps = psmall.tile([128, D], F32, tag="small", name="ps")
                for kc in range(2):
                    nc.tensor.matmul(
                        ps[:], xT[kc][:, si * 128:(si + 1) * 128],
                        w_sb["v"][:, kc, :], start=(kc == 0), stop=False)
                nc.tensor.matmul(ps[:], ones_r[0:1, 0:128], b_sb["v"][0:1, :],
                                 start=False, stop=True)
                nc.scalar.copy(V_sb[:, si, :], ps[:])

            # ---- Phase 3: prior ----
            t0 = cpool.tile([1, H], F32, tag="t0")
            nc.scalar.activation(t0[:], sig_sb[:], AF.Abs)
            nc.vector.tensor_scalar_add(t0[:], t0[:], 1e-6)
            nc.vector.tensor_tensor(t0[:], t0[:], t0[:], op=ALU.mult)
            nc.vector.tensor_scalar_mul(t0[:], t0[:], 2.0)
            inv2s = cpool.tile([1, H], F32, tag="inv2s")
            nc.vector.reciprocal(inv2s[:], t0[:])
            ps_sc = psmall.tile([128, H], F32, tag="small")
            nc.tensor.matmul(ps_sc[:], ones_f[0:1, 0:128], inv2s[:],
                             start=True, stop=True)
            scales = cpool.tile([128, H], F32)
            nc.vector.tensor_copy(scales[:], ps_sc[:])
            prsums = cpool.tile([128, H], F32)
            for h in range(H):
                pr = pattn.tile([128, L], F32, tag="attn", name="pr")
                nc.scalar.activation(pr[:], dist2n_sb[:], AF.Exp,
                                     scale=scales[:, h:h + 1],
                                     accum_out=prsums[:, h:h + 1])
                ssum = ptiny.tile([128, 1], F32, tag="tiny", name="ssum")
                nc.vector.tensor_scalar_add(ssum[:], prsums[:, h:h + 1], 1e-8)
                nc.vector.reciprocal(ssum[:], ssum[:])
                nc.vector.tensor_scalar_mul(pr[:], pr[:], ssum[:])
                nc.sync.dma_start(prior_d[h], pr[:])

            # ---- Phase 4: scores [l,s] -> exp -> normalize -> DMA ----
            sums = cpool.tile([128, 8, H], F32)
            for li in range(8):
                for h in range(H):
                    dc, hp = h // 4, h % 4
                    pss = pmm2.tile([128, L], F32, tag="mm2", name="pss")
                    for sh in range(2):
                        nc.tensor.matmul(
                            pss[:, sh * 512:(sh + 1) * 512],
                            QT[dc][hp * 32:(hp + 1) * 32,
                                   li * 128:(li + 1) * 128],
                            KT[dc][hp * 32:(hp + 1) * 32,
                                   sh * 512:(sh + 1) * 512],
                            start=True, stop=True,
                            tile_position=(hp * 32, 0))
                    e = pattn.tile([128, L], F32, tag="attn", name="e")
                    nc.scalar.activation(e[:], pss[:], AF.Exp, scale=SCALE,
                                         accum_out=sums[:, li, h:h + 1])
                    inv1 = ptiny.tile([128, 1], F32, tag="tiny", name="inv1")
                    nc.vector.reciprocal(inv1[:], sums[:, li, h:h + 1])
                    nc.vector.tensor_scalar_mul(e[:], e[:], inv1[:])
                    nc.sync.dma_start(
                        attn_d[h, li * 128:(li + 1) * 128, :], e[:])

            # invSrows: [8, 1024] = 1/S[h,l] (for normalizing A@V later)
            ps_sr = pmm2.tile([8, L], F32, tag="mm2")
            for li in range(8):
                nc.tensor.transpose(ps_sr[0:8, li * 128:(li + 1) * 128],
                                    sums[:, li, :], eye_sb[:])
            invSrows = cpool.tile([8, L], F32)
            nc.vector.reciprocal(invSrows[:], ps_sr[0:8, :])
            invSrows_r = cpool.tile([8, L], F32R)
            nc.vector.tensor_copy(invSrows_r[:], invSrows[:])

            # ---- Phase 5: scoresT -> expT -> A@V accumulate; normalize ----
            wUT = [cpool.tile([128, L], F32R, tag=f"wUT{g}", name=f"wUT{g}")
                   for g in range(2)]
            for g in range(2):
                # broadcast 1/S[h,l] over each head's 32 partitions via a
                # selection-matrix matmul (K=8)
                psbc = pmm2.tile([128, L], F32, tag="mm2", name="psbc")
                for lh in range(2):
                    nc.tensor.matmul(
                        psbc[:, lh * 512:(lh + 1) * 512],
                        bsel_sb[:, g, :],
                        invSrows_r[:, lh * 512:(lh + 1) * 512],
                        start=True, stop=True)
                invSbc = cpool.tile([128, L], F32, tag=f"invSbc{g}",
                                    name=f"invSbc{g}")
                nc.vector.tensor_copy(invSbc[:], psbc[:])
                for hp in range(4):
                    h = g * 4 + hp
                    eTs = []
                    for si in range(8):
                        pst = pmm2.tile([128, L], F32, tag="mm2", name="pst")
                        for lh in range(2):
                            nc.tensor.matmul(
                                pst[:, lh * 512:(lh + 1) * 512],
                                KT[g][hp * 32:(hp + 1) * 32,
                                      si * 128:(si + 1) * 128],
                                QT[g][hp * 32:(hp + 1) * 32,
                                      lh * 512:(lh + 1) * 512],
                                start=True, stop=True,
                                tile_position=(hp * 32, 0))
                        eT = peT.tile([128, L], F32R, tag="eT", name="eT")
                        nc.scalar.activation(eT[:], pst[:], AF.Exp, scale=SCALE)
                        eTs.append(eT)
                    for lh in range(2):
                        pvh = pav.tile([32, 512], F32, tag="av", name="pvh")
                        for si in range(8):
                            nc.tensor.matmul(
                                pvh[:],
                                V_sb[:, si, h * 32:(h + 1) * 32],
                                eTs[si][:, lh * 512:(lh + 1) * 512],
                                start=(si == 0), stop=(si == 7))
                        nc.vector.tensor_tensor(
                            wUT[g][hp * 32:(hp + 1) * 32,
                                   lh * 512:(lh + 1) * 512],
                            pvh[:],
                            invSbc[hp * 32:(hp + 1) * 32,
                                   lh * 512:(lh + 1) * 512],
                            op=ALU.mult)

            # ---- Phase 6: out = weighted @ Wo + bo, transpose, DMA ----
            outT = [cpool.tile([128, L], F32, tag=f"outT{go}", name=f"outT{go}")
                    for go in range(2)]
            for go in range(2):
                for lh in range(2):
                    ps = psmall.tile([128, 512], F32, tag="small", name="ps")
                    for gi in range(2):
                        nc.tensor.matmul(
                            ps[:], w_sb["o"][:, gi, go * 128:(go + 1) * 128],
                            wUT[gi][:, lh * 512:(lh + 1) * 512],
                            start=(gi == 0), stop=False)
                    nc.tensor.matmul(
                        ps[:], b_sb["o"][0:1, go * 128:(go + 1) * 128],
                        ones_r[0:1, lh * 512:(lh + 1) * 512],
                        start=False, stop=True)
                    nc.scalar.copy(outT[go][:, lh * 512:(lh + 1) * 512], ps[:])
            for li in range(8):
                pso = psmall.tile([128, D], F32, tag="small", name="pso")
                for go in range(2):
                    nc.tensor.transpose(
                        pso[:, go * 128:(go + 1) * 128],
                        outT[go][:, li * 128:(li + 1) * 128], eye_sb[:])
                onat = pattn.tile([128, D], F32, tag="onat", name="onat")
                nc.vector.tensor_copy(onat[:], pso[:])
                nc.sync.dma_start(out_d[li * 128:(li + 1) * 128, :], onat[:])

    nc.compile()
    return nc


def _get_nc():
    if "nc" not in _CACHE:
        _CACHE["nc"] = _build()
    return _CACHE["nc"]


def make_in_maps(inputs):
    x = np.asarray(inputs["x"], dtype=np.float32)
    ws = {k: np.ascontiguousarray(np.asarray(inputs[f"W{k}"], np.float32))
          for k in "qkvo"}
    bs = {k: np.ascontiguousarray(
        np.asarray(inputs[f"b{k}"], np.float32).reshape(1, D))
        for k in "qkvo"}
    sig = np.asarray(inputs["prior_sigma"], np.float32).reshape(1, H)
    eye = np.eye(128, dtype=np.float32)
    ones = np.ones((1, L), dtype=np.float32)
    bsel = np.zeros((8, 2, 128), dtype=np.float32)
    for g in range(2):
        for h in range(8):
            for p in range(128):
                if h == g * 4 + p // 32:
                    bsel[h, g, p] = 1.0
    pos = np.arange(L, dtype=np.float64)

    in_maps = []
    for c in range(N_CORES):
        rows = pos[c * 128:(c + 1) * 128]
        dist2n = (-((rows[:, None] - pos[None, :]) ** 2)).astype(np.float32)
        m = {"x": np.ascontiguousarray(x[c]), "sig": sig, "dist2n": dist2n,
             "eye": eye, "ones": ones, "bsel": bsel}
        for k in "qkvo":
            m[f"W{k}"] = ws[k]
            m[f"b{k}"] = bs[k]
        in_maps.append(m)
    return in_maps


def kernel(**inputs):
    from concourse.bass_utils import run_bass_kernel_spmd

    nc = _get_nc()
    in_maps = make_in_maps(inputs)
    res = run_bass_kernel_spmd(nc, in_maps, list(range(N_CORES)))
    _CACHE["last_results"] = res

    out = np.stack([res.results[c]["out"] for c in range(N_CORES)])
    series = np.stack([res.results[c]["attn"] for c in range(N_CORES)])
    prior_h = np.concatenate(
        [res.results[c]["prior"] for c in range(N_CORES)], axis=1)
    prior = np.broadcast_to(prior_h[None], (B, H, L, L))
    return (out, series, prior)


# revision 9
# speedup vs baseline: 21.1923x; 21.1923x over previous
"""Trainium2 Bass kernel for the AnomalyBlock problem.

Strategy: data-parallel over batch B=8 (one batch element per NeuronCore).
Each core runs the full attention block for its batch element:
  q/k/v/o projections (float32r matmuls, biases folded in as K=1 rank-1
  matmuls against a ones-row), scores in BOTH orientations ([l,s] for the
  normalized attention output, [s,l] for the A@V contraction since the PE
  contracts over the partition dim), softmax via ACT exp with accum_out
  (row sums for free) + DVE per-partition normalize.
prior_attn is batch-broadcast, so only [H,L,L] is unique; its rows are
sharded across the 8 cores (core c computes rows c*128..c*128+127 for all
heads) and the batch dim is broadcast on the host at gather time.
"""
import sys
import numpy as np

sys.path.insert(0, "/opt/trn_rl_repo")

B, L, D, H, DK = 8, 1024, 256, 8, 32
N_CORES = 8
SCALE = 1.0 / np.sqrt(DK)

_CACHE = {}


def _build(repeat=1):
    import concourse.bacc as bacc
    import concourse.mybir as mybir
    import concourse.tile as tile

    F32 = mybir.dt.float32
    F32R = mybir.dt.float32r
    AF = mybir.ActivationFunctionType
    ALU = mybir.AluOpType

    nc = bacc.Bacc("TRN2", target_bir_lowering=False, debug=False,
                   num_devices=N_CORES)

    x_d = nc.dram_tensor("x", [L, D], F32, kind="ExternalInput")
    w_d = {k: nc.dram_tensor(f"W{k}", [D, D], F32, kind="ExternalInput")
           for k in "qkvo"}
    b_d = {k: nc.dram_tensor(f"b{k}", [1, D], F32, kind="ExternalInput")
           for k in "qkvo"}
    sig_d = nc.dram_tensor("sig", [1, H], F32, kind="ExternalInput")
    dist2n_d = nc.dram_tensor("dist2n", [128, L], F32, kind="ExternalInput")
    eye_d = nc.dram_tensor("eye", [128, 128], F32, kind="ExternalInput")
    ones_d = nc.dram_tensor("ones", [1, L], F32, kind="ExternalInput")
    bsel_d = nc.dram_tensor("bsel", [8, 2, 128], F32, kind="ExternalInput")

    attn_d = nc.dram_tensor("attn", [H, L, L], F32, kind="ExternalOutput")
    prior_d = nc.dram_tensor("prior", [H, 128, L], F32, kind="ExternalOutput")
    out_d = nc.dram_tensor("out", [L, D], F32, kind="ExternalOutput")

    with tile.TileContext(nc) as tc:
        with (
            tc.tile_pool(name="const", bufs=1) as cpool,
            tc.tile_pool(name="attnp", bufs=6) as pattn,
            tc.tile_pool(name="eTp", bufs=6) as peT,
            tc.tile_pool(name="tinyp", bufs=8) as ptiny,
            tc.tile_pool(name="mm2", bufs=2, space="PSUM") as pmm2,
            tc.tile_pool(name="avp", bufs=2, space="PSUM") as pav,
            tc.tile_pool(name="smallp", bufs=2, space="PSUM") as psmall,
        ):
            # ---- Phase 0: input DMAs ----
            x_sb = cpool.tile([128, 8, D], F32)
            nc.sync.dma_start(x_sb[:], x_d.rearrange("(a p) d -> p a d", p=128))
            w_sb = {}
            for k in "qkvo":
                w_sb[k] = cpool.tile([128, 2, D], F32R, tag=f"W{k}", name=f"W{k}_sb")
                nc.gpsimd.dma_start(
                    w_sb[k][:], w_d[k].rearrange("(k p) n -> p k n", p=128))
            b_sb = {}
            for k in "qkvo":
                b_sb[k] = cpool.tile([1, D], F32R, tag=f"b{k}", name=f"b{k}_sb")
                nc.gpsimd.dma_start(b_sb[k][:], b_d[k][:])
            sig_sb = cpool.tile([1, H], F32)
            nc.sync.dma_start(sig_sb[:], sig_d[:])
            dist2n_sb = cpool.tile([128, L], F32)
            nc.sync.dma_start(dist2n_sb[:], dist2n_d[:])
            eye_sb = cpool.tile([128, 128], F32)
            nc.sync.dma_start(eye_sb[:], eye_d[:])
            ones_f = cpool.tile([1, L], F32)
            nc.sync.dma_start(ones_f[:], ones_d[:])
            ones_r = cpool.tile([1, L], F32R)
            nc.gpsimd.dma_start(ones_r[:], ones_d[:])
            bsel_sb = cpool.tile([8, 2, 128], F32R)
            nc.gpsimd.dma_start(bsel_sb[:], bsel_d[:])

            # ---- Phases 1-6, optionally repeated for benchmarking ----
            for _rep in range(repeat):
                _emit_compute(nc, tc, cpool, pattn, peT, ptiny, pmm2, pav,
                              psmall, F32, F32R, AF, ALU, x_sb, w_sb, b_sb,
                              sig_sb, dist2n_sb, eye_sb, ones_f, ones_r,
                              bsel_sb, attn_d, prior_d, out_d, _rep)

    nc.compile()
    return nc


def _emit_compute(nc, tc, cpool, pattn, peT, ptiny, pmm2, pav, psmall,
                  F32, F32R, AF, ALU, x_sb, w_sb, b_sb, sig_sb, dist2n_sb,
                  eye_sb, ones_f, ones_r, bsel_sb, attn_d, prior_d, out_d,
                  _rep):
    if True:
        if True:
            # ---- Phase 1: xT = x transposed, [d, l] layout, f32r ----
            xT = [cpool.tile([128, L], F32R, tag=f"xT{dc}", name=f"xT{dc}_{_rep}") for dc in range(2)]
            for li in range(8):
                for dc in range(2):
                    ps = psmall.tile([128, 128], F32, tag="small", name="ps")
                    nc.tensor.transpose(
                        ps[:], x_sb[:, li, dc * 128:(dc + 1) * 128], eye_sb[:])
                    nc.vector.tensor_copy(
                        xT[dc][:, li * 128:(li + 1) * 128], ps[:])

            # ---- Phase 2: projections ----
            # QT/KT: [dout, l] layout (head-major on partitions), f32r
            QT = [cpool.tile([128, L], F32R, tag=f"QT{dc}", name=f"QT{dc}_{_rep}") for dc in range(2)]
            KT = [cpool.tile([128, L], F32R, tag=f"KT{dc}", name=f"KT{dc}_{_rep}") for dc in range(2)]
            for name, wk, bk, dst in (("q", "q", "q", QT), ("k", "k", "k", KT)):
                for dc in range(2):
                    for lh in range(2):
                        ps = psmall.tile([128, 512], F32, tag="small", name="ps")
                        for kc in range(2):
                            nc.tensor.matmul(
                                ps[:],
                                w_sb[wk][:, kc, dc * 128:(dc + 1) * 128],
                                xT[kc][:, lh * 512:(lh + 1) * 512],
                                start=(kc == 0), stop=False)
                        nc.tensor.matmul(
                            ps[:],
                            b_sb[bk][0:1, dc * 128:(dc + 1) * 128],
                            ones_r[0:1, lh * 512:(lh + 1) * 512],
                            start=False, stop=True)
                        nc.scalar.copy(
                            dst[dc][:, lh * 512:(lh + 1) * 512], ps[:])
            # V: natural [s, dout] layout, f32r
            V_sb = cpool.tile([128, 8, D], F32R)
            for si in range(8):
                ps = psmall.tile([128, D], F32, tag="small", name="ps")
                for kc in range(2):
                    nc.tensor.matmul(
                        ps[:], xT[kc][:, si * 128:(si + 1) * 128],
                        w_sb["v"][:, kc, :], start=(kc == 0), stop=False)
                nc.tensor.matmul(ps[:], ones_r[0:1, 0:128], b_sb["v"][0:1, :],
                                 start=False, stop=True)
                nc.scalar.copy(V_sb[:, si, :], ps[:])

            # ---- Phase 3: prior ----
            t0 = cpool.tile([1, H], F32, tag="t0")
            nc.scalar.activation(t0[:], sig_sb[:], AF.Abs)
            nc.vector.tensor_scalar_add(t0[:], t0[:], 1e-6)
            nc.vector.tensor_tensor(t0[:], t0[:], t0[:], op=ALU.mult)
            nc.vector.tensor_scalar_mul(t0[:], t0[:], 2.0)
            inv2s = cpool.tile([1, H], F32, tag="inv2s")
            nc.vector.reciprocal(inv2s[:], t0[:])
            ps_sc = psmall.tile([128, H], F32, tag="small")
            nc.tensor.matmul(ps_sc[:], ones_f[0:1, 0:128], inv2s[:],
                             start=True, stop=True)
            scales = cpool.tile([128, H], F32)
            nc.vector.tensor_copy(scales[:], ps_sc[:])
            prsums = cpool.tile([128, H], F32)
            for h in range(H):
                pr = pattn.tile([128, L], F32, tag="attn", name="pr")
                nc.scalar.activation(pr[:], dist2n_sb[:], AF.Exp,
                                     scale=scales[:, h:h + 1],
                                     accum_out=prsums[:, h:h + 1])
                ssum = ptiny.tile([128, 1], F32, tag="tiny", name="ssum")
                nc.vector.tensor_scalar_add(ssum[:], prsums[:, h:h + 1], 1e-8)
                nc.vector.reciprocal(ssum[:], ssum[:])
                nc.vector.tensor_scalar_mul(pr[:], pr[:], ssum[:])
                nc.sync.dma_start(prior_d[h], pr[:])

            # ---- Phase 4: scores [l,s] -> exp -> normalize -> DMA ----
            sums = cpool.tile([128, 8, H], F32)
            for li in range(8):
                for h in range(H):
                    dc, hp = h // 4, h % 4
                    pss = pmm2.tile([128, L], F32, tag="mm2", name="pss")
                    for sh in range(2):
                        nc.tensor.matmul(
                            pss[:, sh * 512:(sh + 1) * 512],
                            QT[dc][hp * 32:(hp + 1) * 32,
                                   li * 128:(li + 1) * 128],
                            KT[dc][hp * 32:(hp + 1) * 32,
                                   sh * 512:(sh + 1) * 512],
                            start=True, stop=True,
                            tile_position=(hp * 32, 0))
                    e = pattn.tile([128, L], F32, tag="attn", name="e")
                    nc.scalar.activation(e[:], pss[:], AF.Exp, scale=SCALE,
                                         accum_out=sums[:, li, h:h + 1])
                    inv1 = ptiny.tile([128, 1], F32, tag="tiny", name="inv1")
                    nc.vector.reciprocal(inv1[:], sums[:, li, h:h + 1])
                    nc.vector.tensor_scalar_mul(e[:], e[:], inv1[:])
                    nc.sync.dma_start(
                        attn_d[h, li * 128:(li + 1) * 128, :], e[:])

            # invSrows: [8, 1024] = 1/S[h,l] (for normalizing A@V later)
            ps_sr = pmm2.tile([8, L], F32, tag="mm2")
            for li in range(8):
                nc.tensor.transpose(ps_sr[0:8, li * 128:(li + 1) * 128],
                                    sums[:, li, :], eye_sb[:])
            invSrows = cpool.tile([8, L], F32)
            nc.vector.reciprocal(invSrows[:], ps_sr[0:8, :])
            invSrows_r = cpool.tile([8, L], F32R)
            nc.vector.tensor_copy(invSrows_r[:], invSrows[:])

            # ---- Phase 5: scoresT -> expT -> A@V accumulate; normalize ----
            wUT = [cpool.tile([128, L], F32R, tag=f"wUT{g}", name=f"wUT{g}_{_rep}")
                   for g in range(2)]
            for g in range(2):
                # broadcast 1/S[h,l] over each head's 32 partitions via a
                # selection-matrix matmul (K=8)
                psbc = pmm2.tile([128, L], F32, tag="mm2", name="psbc")
                for lh in range(2):
                    nc.tensor.matmul(
                        psbc[:, lh * 512:(lh + 1) * 512],
                        bsel_sb[:, g, :],
                        invSrows_r[:, lh * 512:(lh + 1) * 512],
                        start=True, stop=True)
                invSbc = cpool.tile([128, L], F32, tag=f"invSbc{g}",
                                    name=f"invSbc{g}_{_rep}")
                nc.vector.tensor_copy(invSbc[:], psbc[:])
                for hp in range(4):
                    h = g * 4 + hp
                    eTs = []
                    for si in range(8):
                        pst = pmm2.tile([128, L], F32, tag="mm2", name="pst")
                        for lh in range(2):
                            nc.tensor.matmul(
                                pst[:, lh * 512:(lh + 1) * 512],
                                KT[g][hp * 32:(hp + 1) * 32,
                                      si * 128:(si + 1) * 128],
                                QT[g][hp * 32:(hp + 1) * 32,
                                      lh * 512:(lh + 1) * 512],
                                start=True, stop=True,
                                tile_position=(hp * 32, 0))
                        eT = peT.tile([128, L], F32R, tag="eT", name="eT")
                        nc.scalar.activation(eT[:], pst[:], AF.Exp, scale=SCALE)
                        eTs.append(eT)
                    for lh in range(2):
                        pvh = pav.tile([32, 512], F32, tag="av", name="pvh")
                        for si in range(8):
                            nc.tensor.matmul(
                                pvh[:],
                                V_sb[:, si, h * 32:(h + 1) * 32],
                                eTs[si][:, lh * 512:(lh + 1) * 512],
                                start=(si == 0), stop=(si == 7))
                        nc.vector.tensor_tensor(
                            wUT[g][hp * 32:(hp + 1) * 32,
                                   lh * 512:(lh + 1) * 512],
                            pvh[:],
                            invSbc[hp * 32:(hp + 1) * 32,
                                   lh * 512:(lh + 1) * 512],
                            op=ALU.mult)

            # ---- Phase 6: out = weighted @ Wo + bo, transpose, DMA ----
            outT = [cpool.tile([128, L], F32, tag=f"outT{go}", name=f"outT{go}_{_rep}")
                    for go in range(2)]
            for go in range(2):
                for lh in range(2):
                    ps = psmall.tile([128, 512], F32, tag="small", name="ps")
                    for gi in range(2):
                        nc.tensor.matmul(
                            ps[:], w_sb["o"][:, gi, go * 128:(go + 1) * 128],
                            wUT[gi][:, lh * 512:(lh + 1) * 512],
                            start=(gi == 0), stop=False)
                    nc.tensor.matmul(
                        ps[:], b_sb["o"][0:1, go * 128:(go + 1) * 128],
                        ones_r[0:1, lh * 512:(lh + 1) * 512],
                        start=False, stop=True)
                    nc.scalar.copy(outT[go][:, lh * 512:(lh + 1) * 512], ps[:])
            for li in range(8):
                pso = psmall.tile([128, D], F32, tag="small", name="pso")
                for go in range(2):
                    nc.tensor.transpose(
                        pso[:, go * 128:(go + 1) * 128],
                        outT[go][:, li * 128:(li + 1) * 128], eye_sb[:])
                onat = pattn.tile([128, D], F32, tag="onat", name="onat")
                nc.vector.tensor_copy(onat[:], pso[:])
                nc.sync.dma_start(out_d[li * 128:(li + 1) * 128, :], onat[:])


def _get_nc():
    if "nc" not in _CACHE:
        _CACHE["nc"] = _build()
    return _CACHE["nc"]


def make_in_maps(inputs):
    x = np.asarray(inputs["x"], dtype=np.float32)
    ws = {k: np.ascontiguousarray(np.asarray(inputs[f"W{k}"], np.float32))
          for k in "qkvo"}
    bs = {k: np.ascontiguousarray(
        np.asarray(inputs[f"b{k}"], np.float32).reshape(1, D))
        for k in "qkvo"}
    sig = np.asarray(inputs["prior_sigma"], np.float32).reshape(1, H)
    eye = np.eye(128, dtype=np.float32)
    ones = np.ones((1, L), dtype=np.float32)
    bsel = np.zeros((8, 2, 128), dtype=np.float32)
    for g in range(2):
        for h in range(8):
            for p in range(128):
                if h == g * 4 + p // 32:
                    bsel[h, g, p] = 1.0
    pos = np.arange(L, dtype=np.float64)

    in_maps = []
    for c in range(N_CORES):
        rows = pos[c * 128:(c + 1) * 128]
        dist2n = (-((rows[:, None] - pos[None, :]) ** 2)).astype(np.float32)
        m = {"x": np.ascontiguousarray(x[c]), "sig": sig, "dist2n": dist2n,
             "eye": eye, "ones": ones, "bsel": bsel}
        for k in "qkvo":
            m[f"W{k}"] = ws[k]
            m[f"b{k}"] = bs[k]
        in_maps.append(m)
    return in_maps


def kernel(**inputs):
    from concourse.bass_utils import run_bass_kernel_spmd

    nc = _get_nc()
    in_maps = make_in_maps(inputs)
    res = run_bass_kernel_spmd(nc, in_maps, list(range(N_CORES)))
    _CACHE["last_results"] = res

    out = np.stack([res.results[c]["out"] for c in range(N_CORES)])
    series = np.stack([res.results[c]["attn"] for c in range(N_CORES)])
    prior_h = np.concatenate(
        [res.results[c]["prior"] for c in range(N_CORES)], axis=1)
    prior = np.broadcast_to(prior_h[None], (B, H, L, L))
    return (out, series, prior)
